# revision 12
# baseline (speedup 1.0000x reference)
"""Biaffine span head kernel for Trainium2 (Bass/Tile), SPMD over 8 NeuronCores.

Math (per batch element b):
    Hu   = H @ U                        [L, d]
    a    = H @ w1                       [L]
    c    = H @ w2                       [L]
    bil[l, off] = <Hu[l, :], H[l+off, :]>        off in [0, 30)
    s    = bil + a[l] + b + c[l+off]
    m    = mask[l] * mask_pad[l+off]
    out[l, off] = s if m != 0 else -1e9          [L, 30]

Sharding: data-parallel over batch B=8 across the 8 cores (one batch row per
core); the H@U GEMM and the band loop are fully local per shard, no
collectives.

Per-core schedule (v2): software pipeline over 4 l-blocks of 512.
 - H, U, w are loaded via SWDGE cast-DMAs (f32 DRAM -> f16 SBUF, free RNE
   cast in the DMA datapath) on one FIFO queue, interleaved so U column
   quarters land just in time for the H@U dc-loop.
 - H tiles are transposed on the PE (fp16, grouped 4-to-a-PSUM-bank).
 - HuT = (H@U)^T accumulates over 8 K-chunks per 512-block; a/c come from a
   thin M=2 GEMM and are folded into the band matmul as a 9th contraction
   chunk (lhsT rows [a+b; 1], rhs rows [1; c]), so the band PSUM already
   holds bil + a + b + c.
 - The 30-wide diagonal is extracted via a per-block DRAM bounce whose read
   access pattern has row stride NB+1 (diagonal = affine in DRAM address
   space); the mask fixup and output store run per block, overlapped with
   the next block's GEMMs. Only the last block's fixup is tail latency.
"""

import os
import sys

import numpy as np

for _p in ("/opt/trn_rl_repo",):
    if _p not in sys.path and os.path.isdir(_p):
        sys.path.insert(0, _p)

B = 8
L = 2048
D = 1024
K = 30          # band width (MAX_ANSWER_LEN)
P = 128         # partitions
NB = P + K      # 158: band matmul window
KC = D // P     # 8 contraction chunks
NBLK = 512      # l-block width
LBN = L // NBLK  # 4
TPB = NBLK // P  # 4 l-tiles per block
LT = L // P     # 16 l-tiles
LPAD = L + 32   # padded H^T width (2080)
NEG = -1.0e9

_CACHE = {}


def _build_nc():
    import concourse.bass as bass
    import concourse.tile as tile
    from concourse import bacc, mybir

    f32 = mybir.dt.float32
    f16 = mybir.dt.float16
    i32 = mybir.dt.int32

    nc = bacc.Bacc("TRN2", target_bir_lowering=False, debug=False, num_devices=B)

    H_h = nc.dram_tensor("H", [L, D], f32, kind="ExternalInput")
    mask_h = nc.dram_tensor("mask", [L], i32, kind="ExternalInput")
    U_h = nc.dram_tensor("U", [D, D], f32, kind="ExternalInput")
    w_h = nc.dram_tensor("w", [2 * D], f32, kind="ExternalInput")
    b_h = nc.dram_tensor("b", [1], f32, kind="ExternalInput")
    out_h = nc.dram_tensor("out", [L, K], f32, kind="ExternalOutput")

    band_h = nc.dram_tensor("band_scratch", [LT, P, NB], f32)
    maskf_h = nc.dram_tensor("maskf_scratch", [LPAD], f32)

    ident_h = nc.inline_tensor(np.eye(P, dtype=np.float16), name="ident_const")
    htz_h = nc.inline_tensor(np.zeros((P, KC, 32), dtype=np.float16),
                             name="htz_const")
    ones_h = nc.inline_tensor(np.ones((1, LPAD), dtype=np.float16),
                              name="ones_const")
    zpad_h = nc.inline_tensor(np.zeros((1, 32), dtype=np.float32),
                              name="zpad_const")

    H = H_h.ap()

    def dap(h, off, dims):
        # DRAM access pattern helper: dims = [(stride_elems, count), ...]
        return bass.AP(h, off, [list(d) for d in dims])

    with tile.TileContext(nc) as tc, tc.tile_pool(name="perm", bufs=1) as perm_pool:

        def perm(shape, dtype, name):
            return perm_pool.tile(shape, dtype, name=name, tag=name)

        # ---- persistent SBUF tensors (fp16 datapath) ----
        ident_sb = perm([P, P], f16, name="ident_sb")
        nc.sync.dma_start(ident_sb[:], ident_h.ap())

        U16 = perm([P, KC, D], f16, name="U16")
        HT_all = perm([P, KC, LPAD], f16, name="HT_all")
        HuT = [perm([P, L], f16, name=f"HuT{kk}") for kk in range(KC)]
        w16 = perm([P, KC, 2], f16, name="w16")
        b_sb = perm([1, 1], f32, name="b_sb")
        zpad = perm([1, 32], f32, name="zpad")
        acL = perm([2, L], f16, name="acL")      # row0 = a+b, row1 = ones
        acR = perm([2, LPAD], f16, name="acR")   # row0 = ones, row1 = c (pad 0)

        # constants via gpsimd memsets (cheap, ~150ns each, run before the
        # SWDGE load stream). Engine ops must start at partition 0, so the
        # partition-1 rows (acL ones row, acR zero tail) are layered.
        nc.vector.memset(HT_all[:, :, L:LPAD], 0.0)
        nc.vector.memset(zpad[:], 0.0)
        nc.vector.memset(acL[0:2, :], 1.0)          # row0 overwritten per block
        nc.vector.memset(acR[0:2, :], 1.0)
        nc.vector.memset(acR[0:2, L:LPAD], 0.0)     # zero both tails...
        nc.vector.memset(acR[0:1, L:LPAD], 1.0)     # ...restore ones-row tail

        # mask -> f32 -> DRAM bounce (for the windowed gathers in fixup)
        with tc.tile_pool(name="mcast", bufs=1) as mcast_pool, \
             tc.tile_pool(name="hstage", bufs=4) as h16_pool, \
             tc.tile_pool(name="trpsum", bufs=2, space="PSUM") as trps, \
             tc.tile_pool(name="hupsum", bufs=4, space="PSUM") as hups, \
             tc.tile_pool(name="bandpsum", bufs=1, space="PSUM") as bps, \
             tc.tile_pool(name="acpsum", bufs=1, space="PSUM") as acps, \
             tc.tile_pool(name="bandsb", bufs=2) as bsb_pool, \
             tc.tile_pool(name="fix", bufs=2) as fix_pool:

            # ---- input loads ----
            # ONE SWDGE cast-load FIFO in exact need-order: H block 0 first
            # (gates the transposes), then U row chunks (gate gemm1(0),
            # consumed kk-outer at arrival pace), then H blocks 1-3. A
            # second parallel queue just steals SDMA round-robin slots from
            # this stream, so everything big rides one queue.
            hstages = {}

            def load_hblock(lb):
                hb = h16_pool.tile([P, TPB, D], f16, name="h16b", tag="h16b")
                nc.gpsimd.dma_start(
                    hb[:], dap(H_h, lb * NBLK * D, [(D, P), (P * D, TPB), (1, D)])
                )
                hstages[lb] = hb

            load_hblock(0)
            for kk in range(KC):
                nc.gpsimd.dma_start(
                    U16[:, kk, :], dap(U_h, kk * P * D, [(D, P), (1, D)])
                )
            load_hblock(1)
            load_hblock(2)
            load_hblock(3)

            # small stragglers: issued FIRST on the (otherwise idle) Scalar
            # HWDGE ring so they land in ~1-2us. v2 had these at the back of
            # the Sync FIFO behind the h0 tiles + bounce writes: w16 landed
            # at ~42us and, ac_gemm(0) being before gemm1(0) in the in-order
            # PE stream, stalled the whole GEMM core until then.
            w_s = mcast_pool.tile([P, KC, 2], f32, name="w_s")
            nc.scalar.dma_start(w_s[:, :, 0], dap(w_h, 0, [(1, P), (P, KC)]))
            nc.scalar.dma_start(w_s[:, :, 1], dap(w_h, D, [(1, P), (P, KC)]))
            nc.vector.tensor_copy(w16[:], w_s[:])
            nc.scalar.dma_start(b_sb[:], dap(b_h, 0, [(1, 1), (1, 1)]))
            m_i = mcast_pool.tile([P, LT], i32, name="m_i")
            nc.scalar.dma_start(m_i[:], dap(mask_h, 0, [(LT, P), (1, LT)]))
            m_f = mcast_pool.tile([P, LT], f32, name="m_f")
            nc.vector.tensor_copy(m_f[:], m_i[:])
            nc.scalar.dma_start(dap(maskf_h, 0, [(LT, P), (1, LT)]), m_f[:])
            nc.scalar.dma_start(dap(maskf_h, L, [(32, 1), (1, 32)]), zpad[0:1, :])

            # ---- per-block phases ----
            def transposes(lb):
                # two k-chunks share one PSUM bank -> one [P,1024]-f16 vector
                # evac per pair (halves evac instruction count + rate)
                j0 = lb * NBLK

                def src(i, kk):
                    return hstages[lb][:, i, kk * P:(kk + 1) * P]
                for kp in range(KC // 2):
                    tp = trps.tile([P, 2, NBLK], f16, name="tp", tag="tp")
                    for kk2 in range(2):
                        kk = kp * 2 + kk2
                        for i in range(TPB):
                            nc.tensor.matmul(
                                tp[:, kk2, i * P:(i + 1) * P],
                                lhsT=src(i, kk),
                                rhs=ident_sb[:],
                                is_transpose=True,
                                start=(kk2 == 0 and i == 0),
                                stop=(kk2 == 1 and i == TPB - 1),
                            )
                    nc.vector.tensor_copy(
                        HT_all[:, kp * 2:kp * 2 + 2, j0:j0 + NBLK], tp[:]
                    )
                del hstages[lb]

            def ac_gemm(lb):
                j0 = lb * NBLK
                acp = acps.tile([2, NBLK], f32, name="acp", tag="acp")
                for kk in range(KC):
                    nc.tensor.matmul(
                        acp[:],
                        lhsT=w16[:, kk, :],
                        rhs=HT_all[:, kk, j0:j0 + NBLK],
                        start=(kk == 0),
                        stop=(kk == KC - 1),
                    )
                nc.vector.tensor_scalar_add(
                    acL[0:1, j0:j0 + NBLK], acp[0:1, :], b_sb[0:1, 0:1],
                )
                # c goes to partition 1 of acR: engines can't write there, so
                # stage both rows (base partition 0) and DMA row 1 across
                st = fix_pool.tile([2, NBLK], f16, name="acst", tag="acst")
                nc.vector.tensor_copy(st[:], acp[:])
                nc.sync.dma_start(acR[1:2, j0:j0 + NBLK], st[1:2, :])

            def gemm1(lb, dcs):
                j0 = lb * NBLK
                for dc in dcs:
                    hp = hups.tile([P, NBLK], f32, name="hp", tag="hp")
                    for kk in range(KC):
                        nc.tensor.matmul(
                            hp[:],
                            lhsT=U16[:, kk, dc * P:(dc + 1) * P],
                            rhs=HT_all[:, kk, j0:j0 + NBLK],
                            start=(kk == 0),
                            stop=(kk == KC - 1),
                        )
                    nc.vector.tensor_copy(HuT[dc][:, j0:j0 + NBLK], hp[:])

            def gemm1_kkouter(lb, dcs):
                # kk-outer over a quad of dc accumulators: each U row chunk
                # is consumed the moment its cast-DMA lands, so gemm1(0)
                # paces the SWDGE stream instead of stalling on kk=7.
                j0 = lb * NBLK
                hps = [
                    (dc, hups.tile([P, NBLK], f32, name="hp", tag="hp"))
                    for dc in dcs
                ]
                for kk in range(KC):
                    for dc, hp in hps:
                        nc.tensor.matmul(
                            hp[:],
                            lhsT=U16[:, kk, dc * P:(dc + 1) * P],
                            rhs=HT_all[:, kk, j0:j0 + NBLK],
                            start=(kk == 0),
                            stop=(kk == KC - 1),
                        )
                for dc, hp in hps:
                    nc.vector.tensor_copy(HuT[dc][:, j0:j0 + NBLK], hp[:])

            def band(lb, bsb, tiles, dma):
                for i in tiles:
                    l0 = (lb * TPB + i) * P
                    bp = bps.tile([P, NB], f32, name="bp", tag="bp")
                    for kk in range(KC):
                        nc.tensor.matmul(
                            bp[:],
                            lhsT=HuT[kk][:, l0:l0 + P],
                            rhs=HT_all[:, kk, l0:l0 + NB],
                            start=(kk == 0),
                            stop=False,
                        )
                    # 9th chunk: + a[l] + b + c[j]
                    nc.tensor.matmul(
                        bp[:],
                        lhsT=acL[:, l0:l0 + P],
                        rhs=acR[:, l0:l0 + NB],
                        start=False,
                        stop=True,
                    )
                    nc.vector.tensor_copy(bsb[:, i, :], bp[:])
                if dma:
                    nc.sync.dma_start(
                        dap(band_h, lb * TPB * P * NB,
                            [(NB, P), (P * NB, TPB), (1, NB)]),
                        bsb[:],
                    )

            def fixup(lb):
                # diagonal band extraction: affine gather from the DRAM bounce
                bd = fix_pool.tile([P, TPB, K], f32, name="bd", tag="bd")
                nc.sync.dma_start(
                    bd[:],
                    dap(band_h, lb * TPB * P * NB,
                        [(NB + 1, P), (P * NB, TPB), (1, K)]),
                )
                md = fix_pool.tile([P, TPB, K], f32, name="md", tag="md")
                nc.scalar.dma_start(
                    md[:], dap(maskf_h, lb * NBLK, [(1, P), (P, TPB), (1, K)])
                )
                # broadcast view: md[:, :, 0] (= mask[l]) repeated along K
                mc_bc = bass.AP(md.tensor, md.offset,
                                list(md.ap[:1]) + [[K, TPB], [0, K]])
                m_all = fix_pool.tile([P, TPB, K], f32, name="m_all", tag="m_all")
                nc.vector.tensor_mul(m_all[:], md[:], mc_bc)
                f3 = fix_pool.tile([P, TPB, K], f32, name="f3", tag="f3")
                nc.vector.tensor_mul(f3[:], bd[:], m_all[:])
                f4 = fix_pool.tile([P, TPB, K], f32, name="f4", tag="f4")
                nc.vector.tensor_scalar(
                    f4[:], in0=m_all[:], scalar1=1.0, scalar2=-NEG,
                    op0=mybir.AluOpType.subtract, op1=mybir.AluOpType.mult,
                )
                o = fix_pool.tile([P, TPB, K], f32, name="o_t", tag="o_t")
                nc.vector.tensor_add(o[:], f3[:], f4[:])
                nc.scalar.dma_start(
                    dap(out_h, lb * NBLK * K, [(K, P), (P * K, TPB), (1, K)]),
                    o[:],
                )

            # ---- pipeline ----
            # ~40 throwaway matmuls on the identity while the first H tiles
            # stream in: keeps the PE busy through the HAM activity window so
            # the real work runs at 2.4 GHz instead of the cold 1.2 GHz
            for wu in range(5):
                wp = hups.tile([P, NBLK], f32, name="hp", tag="hp")
                for r in range(4):
                    nc.tensor.matmul(
                        wp[:, 0:P],
                        lhsT=ident_sb[:],
                        rhs=ident_sb[:],
                        start=(r == 0),
                        stop=(r == 3),
                    )
            transposes(0)
            bsbs = [
                bsb_pool.tile([P, TPB, NB], f32, name="bsb", tag="bsb")
                for _ in range(2)
            ]
            # PE stream ordered by data arrival: gemm1(0) kk-outer paces the
            # U chunk stream; ac_gemm(lb) sits AFTER the gemm so the slow
            # tiny w-load never gates the GEMM core; band tiles 0-2 (which
            # need only block lb) fill the wait for H block lb+1, tile 3
            # (spills 30 cols into lb+1) runs right after transposes(lb+1).
            gemm1_kkouter(0, range(4))
            gemm1_kkouter(0, range(4, KC))
            ac_gemm(0)
            band(0, bsbs[0], range(3), dma=False)
            for lb in range(1, LBN):
                transposes(lb)
                ac_gemm(lb)
                band(lb - 1, bsbs[(lb - 1) % 2], [3], dma=True)
                gemm1(lb, range(KC))
                band(lb, bsbs[lb % 2], range(3), dma=False)
                fixup(lb - 1)
            band(LBN - 1, bsbs[(LBN - 1) % 2], [3], dma=True)
            fixup(LBN - 1)

    nc.compile()
    return nc


def get_nc():
    if "nc" not in _CACHE:
        _CACHE["nc"] = _build_nc()
    return _CACHE["nc"]


def kernel(H, attention_mask, U, w, b):
    from concourse.bass_utils import run_bass_kernel_spmd

    nc = get_nc()
    H = np.asarray(H, dtype=np.float32)
    attention_mask = np.asarray(attention_mask, dtype=np.int32)
    U_np = np.ascontiguousarray(np.asarray(U, dtype=np.float32))
    w_np = np.ascontiguousarray(np.asarray(w, dtype=np.float32).reshape(-1))
    b_np = np.ascontiguousarray(np.asarray(b, dtype=np.float32).reshape(-1))

    in_maps = []
    for i in range(B):
        in_maps.append({
            "H": np.ascontiguousarray(H[i]),
            "mask": np.ascontiguousarray(attention_mask[i]),
            "U": U_np,
            "w": w_np,
            "b": b_np,
        })
    res = run_bass_kernel_spmd(nc, in_maps, list(range(B)))
    return np.stack([res.results[i]["out"] for i in range(B)], axis=0)

